# revision 23
# baseline (speedup 1.0000x reference)
"""Trainium2 Bass kernel for the Darcy64 residual (dense stencil + BC extraction).

Contract: kernel(**inputs) takes the FULL inputs from setup_inputs()
(x0_pred [2048,2,64,64] f32, compute_bc scalar) and returns the FULL
output [2048,3,64,64] f32 (or [2048,1,64,64] if compute_bc is falsy).

Strategy: pure data parallel over 8 NeuronCores (256 samples each),
128 samples per tile on SBUF partitions, each sample's [64,64] grid
flattened along the free dim.  v3 design notes:

  * All compute in bf16 so DVE tensor_tensor runs in the 2x_1p perf mode.
    scalar_tensor_tensor has NO 2x uop (measured 1x on HW), so every
    interior op must be a plain tensor_tensor: the scalar constants are
    folded away by prescaling x1 by -C/4 on the ACT engine (the residual
    is linear in x1, so the scale propagates through every x1-derived
    term), plus ACT-precomputed 2*x1' and 4*(x0+1) operand tensors.
      res = 4(x0+1)*S2' + A0*P0' + A1*P1' - f_s   (primes: from -C/4*x1)
  * j-direction stencils write +1-shifted buffers (buf[k] = val[k-1]) so
    every big DVE op keeps 4-byte-aligned APs (2x mode requires that).
    The two parity-crossing merges (q1 = t_j - 2x1', V = A1*P1') run on
    GpSimd, which has no alignment-gated perf modes.
  * Inputs are cast fp32->bf16 during the DMA load (SWDGE, prefetched
    up front); the residual and the dense-zero BC ch2 plane are cast
    bf16->fp32 during the store.

Boundary handling (d = 1/64, flat index = i*64 + j):
  First-derivative end rows/cols use the *first*-end coefficients at both
  ends, which flips the sign of the last row/col; the flip cancels in the
  products A*P and makes the BC extraction scale uniform.
  f_s is zero except +10 on grid [0:8,0:8] and -10 on [56:64,56:64].
  BC ch1 rows = (BC_SCALE/C4)*P0' rows; ch2 cols = -(BC_SCALE/C4)*P1'.
  ch1 relies on the runtime's pre-zeroed output buffers (only rows 0,63
  stored); ch2 stores a dense bf16 zero plane with the two columns set.
"""

import sys
from contextlib import ExitStack

import numpy as np

sys.path.insert(0, "/opt/trn_rl_repo")

import concourse.bass as bass  # noqa: E402
import concourse.tile as tile  # noqa: E402
from concourse import mybir  # noqa: E402

N_CORES = 8
B = 2048
S_PER_CORE = B // N_CORES  # 256
P = 128                    # samples per tile (partition dim)
N = 64
G = N * N                  # 4096
C = 39.1 * float(N * N)    # 39.1 / d^2 = 160153.6
C4 = C / 4.0
BC_SCALE = 1.7 * (N / 2.0)  # 1.7/(2d) = 54.4
BC1_SCALE = BC_SCALE / C4   # ch1 scale on the prescaled P0'
BC2_SCALE = -BC_SCALE / C4  # ch2 scale on the prescaled P1'

F32 = mybir.dt.float32
BF16 = mybir.dt.bfloat16
ALU = mybir.AluOpType
COPY = mybir.ActivationFunctionType.Copy


def _fix_first_rows(nc, dstv, srcv):
    """First-derivative one-sided ends on rows 0 and 63 ([p, 64, 64] views).
    First-end coefficients mirrored at the far end (sign flip there cancels
    in products / makes the BC scale uniform)."""
    for (r0, r1, r2) in ((0, 1, 2), (N - 1, N - 2, N - 3)):
        d = dstv[:, r0:r0 + 1, :]
        f0 = srcv[:, r0:r0 + 1, :]
        f1 = srcv[:, r1:r1 + 1, :]
        f2 = srcv[:, r2:r2 + 1, :]
        nc.vector.scalar_tensor_tensor(d, f0, -3.0, f2, ALU.mult, ALU.subtract)
        nc.vector.scalar_tensor_tensor(d, f1, 4.0, d, ALU.mult, ALU.add)


def _fix_second_rows(nc, dstv, srcv):
    for (r0, r1, r2, r3) in ((0, 1, 2, 3), (N - 1, N - 2, N - 3, N - 4)):
        d = dstv[:, r0:r0 + 1, :]
        f0 = srcv[:, r0:r0 + 1, :]
        f1 = srcv[:, r1:r1 + 1, :]
        f2 = srcv[:, r2:r2 + 1, :]
        f3 = srcv[:, r3:r3 + 1, :]
        nc.vector.scalar_tensor_tensor(d, f0, 2.0, f3, ALU.mult, ALU.subtract)
        nc.vector.scalar_tensor_tensor(d, f1, -5.0, d, ALU.mult, ALU.add)
        nc.vector.scalar_tensor_tensor(d, f2, 4.0, d, ALU.mult, ALU.add)


def _fix_first_cols(nc, dstv, srcv):
    for (c0, c1, c2) in ((0, 1, 2), (N - 1, N - 2, N - 3)):
        d = dstv[:, :, c0:c0 + 1]
        f0 = srcv[:, :, c0:c0 + 1]
        f1 = srcv[:, :, c1:c1 + 1]
        f2 = srcv[:, :, c2:c2 + 1]
        nc.vector.scalar_tensor_tensor(d, f0, -3.0, f2, ALU.mult, ALU.subtract)
        nc.vector.scalar_tensor_tensor(d, f1, 4.0, d, ALU.mult, ALU.add)


def _fix_second_cols(nc, dstv, srcv):
    for (c0, c1, c2, c3) in ((0, 1, 2, 3), (N - 1, N - 2, N - 3, N - 4)):
        d = dstv[:, :, c0:c0 + 1]
        f0 = srcv[:, :, c0:c0 + 1]
        f1 = srcv[:, :, c1:c1 + 1]
        f2 = srcv[:, :, c2:c2 + 1]
        f3 = srcv[:, :, c3:c3 + 1]
        nc.vector.scalar_tensor_tensor(d, f0, 2.0, f3, ALU.mult, ALU.subtract)
        nc.vector.scalar_tensor_tensor(d, f1, -5.0, d, ALU.mult, ALU.add)
        nc.vector.scalar_tensor_tensor(d, f2, 4.0, d, ALU.mult, ALU.add)


def _emit_tile(tc, out_ap, s0, bufs, shared, last_tile):
    """Emit one 128-sample tile starting at sample s0 (within this core).
    x1s holds -C4*x1 (prescaled by ACT, in place over the loaded x1)."""
    nc = tc.nc
    x0b, x1s, a0c, p0c, q0, pj0, pj1, q1, v, bc1, bc2c = bufs
    x1d, w4, tjq, resf, x1f, x_ap = shared

    x0v = x0b[:].rearrange("p (h w) -> p h w", h=N)

    # ---- x1 load (HWDGE, fp32 -> shared staging; SWDGE descriptor-ring
    # traffic interferes with DVE streaming, HWDGE does not) ---------------
    nc.sync.dma_start(
        out=x1f[:],
        in_=x_ap[s0:s0 + P, 1].rearrange("s h w -> s (h w)"))

    # ---- ACT prescales (cast fp32->bf16 fused into the scale) -----------
    # x1' = -C4 * x1; x1d = 2*x1'; w4 = 4*(x0+1)
    nc.scalar.activation(x1s[:], x1f[:], COPY, bias=0.0, scale=-C4)
    x1v = x1s[:].rearrange("p (h w) -> p h w", h=N)
    nc.scalar.activation(x1d[:], x1s[:], COPY, bias=0.0, scale=2.0)
    nc.scalar.activation(w4[:], x0b[:], COPY, bias=4.0, scale=4.0)

    # ---- j-direction stencils into +1-shifted buffers (APs all even) ----
    # tjq[k] = x1'[k] + x1'[k-2]  (t_j shifted +1); pj1[k] = P1'[k-1]
    nc.vector.tensor_add(tjq[:, 2:G], x1s[:, 2:G], x1s[:, 0:G - 2])
    nc.vector.tensor_sub(pj1[:, 2:G], x1s[:, 2:G], x1s[:, 0:G - 2])
    pj1n = pj1[:, 1:1 + G].rearrange("p (h w) -> p h w", h=N)
    _fix_first_cols(nc, pj1n, x1v)
    # q1 = t_j - 2*x1' at natural positions (parity crossing: odd APs, 1x).
    # GpSimd would run this in parallel, but a streaming GpSimd tensor op
    # slows concurrent DVE ops ~4x (SBUF interference), so DVE-1x is faster.
    nc.vector.tensor_sub(q1[:, 1:G - 1], tjq[:, 2:G], x1d[:, 1:G - 1])

    nc.vector.tensor_sub(pj0[:, 2:G], x0b[:, 2:G], x0b[:, 0:G - 2])
    pj0n = pj0[:, 1:1 + G].rearrange("p (h w) -> p h w", h=N)
    _fix_first_cols(nc, pj0n, x0v)
    # V = A1 * P1' on the +1-shifted grid (all-even APs keep 2x mode)
    nc.vector.tensor_mul(v[:], pj0[:], pj1[:])

    # ---- i-direction stencils ------------------------------------------
    nc.vector.tensor_sub(p0c[:, N:G - N], x1s[:, 2 * N:G], x1s[:, 0:G - 2 * N])
    p0v = p0c[:].rearrange("p (h w) -> p h w", h=N)
    _fix_first_rows(nc, p0v, x1v)
    nc.vector.tensor_sub(a0c[:, N:G - N], x0b[:, 2 * N:G], x0b[:, 0:G - 2 * N])
    a0v = a0c[:].rearrange("p (h w) -> p h w", h=N)
    _fix_first_rows(nc, a0v, x0v)

    # BC ch1: rows 0,63 of P0' -> fp32 bc1, stored dense (2 rows/sample).
    nc.scalar.activation(bc1[:], p0v[:, 0:N:N - 1, :], COPY,
                         bias=0.0, scale=BC1_SCALE)
    nc.sync.dma_start(out=out_ap[s0:s0 + P, 1, 0:N:N - 1, :], in_=bc1[:])
    # BC ch2: cols 0,63 of P1' compacted to fp32 [P,64,2]; only those two
    # columns are stored (the rest of the plane relies on the runtime's
    # pre-zeroed output buffer, same as ch1's interior rows).
    nc.scalar.activation(bc2c[:], pj1n[:, :, 0:N:N - 1], COPY,
                         bias=0.0, scale=BC2_SCALE)
    # SWDGE (GpSimd is otherwise idle): generating the 8K four-byte
    # descriptors on HWDGE (~12us) would block the sync queue and delay
    # the next tile's x1 load
    for c in (0, 1):
        nc.gpsimd.dma_start(out=out_ap[s0:s0 + P, 2, :, c * (N - 1)],
                            in_=bc2c[:, :, c])

    # U = A0 * P0' (in place over a0c)
    nc.vector.tensor_mul(a0c[:], a0c[:], p0c[:])

    # t_i and q0 (x1' channel)
    nc.vector.tensor_add(q0[:, N:G - N], x1s[:, 2 * N:G], x1s[:, 0:G - 2 * N])
    nc.vector.tensor_sub(q0[:, N:G - N], q0[:, N:G - N], x1d[:, N:G - N])
    q0v = q0[:].rearrange("p (h w) -> p h w", h=N)
    _fix_second_rows(nc, q0v, x1v)
    q1v = q1[:].rearrange("p (h w) -> p h w", h=N)
    _fix_second_cols(nc, q1v, x1v)

    # ---- combine (in-place over q0) ------------------------------------
    nc.vector.tensor_add(q0[:], q0[:], q1[:])   # S2'
    nc.vector.tensor_mul(q0[:], w4[:], q0[:])   # 4(x0+1)*S2'
    nc.vector.tensor_add(q0[:], q0[:], a0c[:])  # + A0*P0'
    # + A1*P1' back on the natural grid (parity crossing: odd src, 1x)
    nc.vector.tensor_add(q0[:], q0[:], v[:, 1:1 + G])

    # source-term corners: res[0:8,0:8] -= 10 ; res[56:64,56:64] += 10
    nc.scalar.activation(q0v[:, 0:8, 0:8], q0v[:, 0:8, 0:8], COPY,
                         bias=-10.0, scale=1.0)
    nc.scalar.activation(q0v[:, N - 8:N, N - 8:N], q0v[:, N - 8:N, N - 8:N],
                         COPY, bias=10.0, scale=1.0)
    # cast bf16 -> fp32 on ACT, store via HWDGE (keeps SWDGE descriptor
    # generation off the Q7 cores that run the GpSimd tensor ops)
    nc.scalar.activation(resf[:], q0[:], COPY, bias=0.0, scale=1.0)
    nc.scalar.dma_start(
        out=out_ap[s0:s0 + P, 0].rearrange("s h w -> s (h w)"), in_=resf[:])


_WAITSPLIT_N = [0]


def _split_excess_waits(nc, max_waits=1):
    """Engine compute-instruction ISA structs hold only one sync-wait slot;
    Tile can assign several at cross-engine join points ("Too many sync wait
    commands" at codegen).  Move all but one wait onto InstNoOp carriers
    inserted just before, on the same engine."""
    keep = (mybir.InstEventSemaphore,
            mybir.InstCall, mybir.InstUnconditionalBranch, mybir.InstNoOp,
            mybir.InstRegisterMove, mybir.InstISA)
    for f in nc.m.functions:
        for b in f.blocks:
            new_insts = []
            for inst in b.instructions:
                si = inst.sync_info
                if (si is not None and si.on_wait and len(si.on_wait) > max_waits
                        and not isinstance(inst, keep)
                        and getattr(inst, "engine", None) is not None):
                    waits = list(si.on_wait)
                    excess, rest = waits[:-max_waits], waits[-max_waits:]
                    for w in excess:
                        _WAITSPLIT_N[0] += 1
                        nop = mybir.InstNoOp(
                            name=f"waitsplit_{_WAITSPLIT_N[0]}",
                            engine=inst.engine,
                            sync_info=mybir.SyncInfo(on_wait=[w], on_update=[]),
                            bass_nofuse=True,
                        )
                        new_insts.append(nop)
                    inst.sync_info = mybir.SyncInfo(on_wait=rest,
                                                    on_update=list(si.on_update))
                new_insts.append(inst)
            b.instructions = new_insts


def build_bass(split_waits=True):
    nc = bass.Bass()
    x = nc.declare_dram_parameter("x", [S_PER_CORE, 2, N, N], F32,
                                  isOutput=False)
    out = nc.declare_dram_parameter("out", [S_PER_CORE, 3, N, N], F32,
                                    isOutput=True)
    with tile.TileContext(nc) as tc:
        with ExitStack() as ctx:
            pool = ctx.enter_context(tc.tile_pool(name="scratch", bufs=1))
            n_tiles = S_PER_CORE // P

            # shared across tiles (short lifetimes / serialization is cheap)
            x1d = pool.tile([P, G], BF16, tag="x1d", name="x1d")
            w4 = pool.tile([P, G], BF16, tag="w4", name="w4")
            tjq = pool.tile([P, G + 2], BF16, tag="tjq", name="tjq")
            resf = pool.tile([P, G], F32, tag="resf", name="resf")
            x1f = pool.tile([P, G], F32, tag="x1f", name="x1f")
            shared = (x1d, w4, tjq, resf, x1f, x[:])

            sets = []
            for t in range(2):
                x0b = pool.tile([P, G], BF16, tag=f"x0b{t}", name=f"x0b{t}")
                x1s = pool.tile([P, G], BF16, tag=f"x1s{t}", name=f"x1s{t}")
                a0c = pool.tile([P, G], BF16, tag=f"a0c{t}", name=f"a0c{t}")
                p0c = pool.tile([P, G], BF16, tag=f"p0c{t}", name=f"p0c{t}")
                q0 = pool.tile([P, G], BF16, tag=f"q0_{t}", name=f"q0_{t}")
                pj0 = pool.tile([P, G + 2], BF16, tag=f"pj0_{t}",
                                name=f"pj0_{t}")
                pj1 = pool.tile([P, G + 2], BF16, tag=f"pj1_{t}",
                                name=f"pj1_{t}")
                q1 = pool.tile([P, G], BF16, tag=f"q1_{t}", name=f"q1_{t}")
                v = pool.tile([P, G + 2], BF16, tag=f"v{t}", name=f"v{t}")
                bc1 = pool.tile([P, 2, N], F32, tag=f"bc1_{t}", name=f"bc1_{t}")
                bc2c = pool.tile([P, N, 2], F32, tag=f"bc2c{t}",
                                 name=f"bc2c{t}")
                sets.append((x0b, x1s, a0c, p0c, q0, pj0, pj1, q1, v, bc1,
                             bc2c))

            # prefetch the x0 channel up front (fp32 -> bf16 SWDGE cast,
            # finishes before the DVE chain is in full swing); x1 is loaded
            # per tile via HWDGE into the fp32 staging buffer
            for it in range(n_tiles):
                nc.gpsimd.dma_start(
                    out=sets[it % 2][0][:],
                    in_=x[:][it * P:(it + 1) * P, 0].rearrange(
                        "s h w -> s (h w)"))

            for it in range(n_tiles):
                _emit_tile(tc, out[:], it * P, sets[it % 2], shared,
                           last_tile=(it == n_tiles - 1))
    if split_waits:
        _split_excess_waits(nc)
    return nc


_NC = None


def _get_nc():
    global _NC
    if _NC is None:
        _NC = build_bass()
    return _NC


def _axon_device_reset():
    """Recover a wedged NeuronCore (NRT_EXEC_UNIT_UNRECOVERABLE) via the
    axon client's reset entry point."""
    try:
        import ctypes

        import jax

        jax.devices()
        lib = ctypes.CDLL("/opt/axon/libaxon_pjrt.so")
        lib.axon_reset.restype = ctypes.c_int64
        return int(lib.axon_reset()) == 0
    except Exception:
        return False


def kernel(x0_pred, compute_bc=1, **_):
    from concourse.bass_utils import run_bass_kernel_spmd

    x = np.ascontiguousarray(np.asarray(x0_pred), dtype=np.float32)
    assert x.shape == (B, 2, N, N), x.shape
    nc = _get_nc()
    shards = x.reshape(N_CORES, S_PER_CORE, 2, N, N)
    in_maps = [{"x": shards[i]} for i in range(N_CORES)]
    try:
        res = run_bass_kernel_spmd(nc, in_maps, list(range(N_CORES)))
    except Exception:
        if not _axon_device_reset():
            raise
        res = run_bass_kernel_spmd(nc, in_maps, list(range(N_CORES)))
    full = np.concatenate([res.results[i]["out"] for i in range(N_CORES)],
                          axis=0)
    if not int(np.asarray(compute_bc)):
        return full[:, :1]
    return full


# revision 24
# speedup vs baseline: 1.0871x; 1.0871x over previous
"""Trainium2 Bass kernel for the Darcy64 residual (dense stencil + BC extraction).

Contract: kernel(**inputs) takes the FULL inputs from setup_inputs()
(x0_pred [2048,2,64,64] f32, compute_bc scalar) and returns the FULL
output [2048,3,64,64] f32 (or [2048,1,64,64] if compute_bc is falsy).

Strategy: pure data parallel over 8 NeuronCores (256 samples each),
128 samples per tile on SBUF partitions, each sample's [64,64] grid
flattened along the free dim.  v3 design notes:

  * All compute in bf16 so DVE tensor_tensor runs in the 2x_1p perf mode.
    scalar_tensor_tensor has NO 2x uop (measured 1x on HW), so every
    interior op must be a plain tensor_tensor: the scalar constants are
    folded away by prescaling x1 by -C/4 on the ACT engine (the residual
    is linear in x1, so the scale propagates through every x1-derived
    term), plus ACT-precomputed 2*x1' and 4*(x0+1) operand tensors.
      res = 4(x0+1)*S2' + A0*P0' + A1*P1' - f_s   (primes: from -C/4*x1)
  * j-direction stencils write +1-shifted buffers (buf[k] = val[k-1]) so
    every big DVE op keeps 4-byte-aligned APs (2x mode requires that).
    The two parity-crossing merges (q1 = t_j - 2x1', V = A1*P1') run on
    GpSimd, which has no alignment-gated perf modes.
  * Inputs are cast fp32->bf16 during the DMA load (SWDGE, prefetched
    up front); the residual and the dense-zero BC ch2 plane are cast
    bf16->fp32 during the store.

Boundary handling (d = 1/64, flat index = i*64 + j):
  First-derivative end rows/cols use the *first*-end coefficients at both
  ends, which flips the sign of the last row/col; the flip cancels in the
  products A*P and makes the BC extraction scale uniform.
  f_s is zero except +10 on grid [0:8,0:8] and -10 on [56:64,56:64].
  BC ch1 rows = (BC_SCALE/C4)*P0' rows; ch2 cols = -(BC_SCALE/C4)*P1'.
  ch1 relies on the runtime's pre-zeroed output buffers (only rows 0,63
  stored); ch2 stores a dense bf16 zero plane with the two columns set.
"""

import sys
from contextlib import ExitStack

import numpy as np

sys.path.insert(0, "/opt/trn_rl_repo")

import concourse.bass as bass  # noqa: E402
import concourse.tile as tile  # noqa: E402
from concourse import mybir  # noqa: E402

N_CORES = 8
B = 2048
S_PER_CORE = B // N_CORES  # 256
P = 128                    # samples per tile (partition dim)
N = 64
G = N * N                  # 4096
C = 39.1 * float(N * N)    # 39.1 / d^2 = 160153.6
C4 = C / 4.0
BC_SCALE = 1.7 * (N / 2.0)  # 1.7/(2d) = 54.4
BC1_SCALE = BC_SCALE / C4   # ch1 scale on the prescaled P0'
BC2_SCALE = -BC_SCALE / C4  # ch2 scale on the prescaled P1'

F32 = mybir.dt.float32
BF16 = mybir.dt.bfloat16
ALU = mybir.AluOpType
COPY = mybir.ActivationFunctionType.Copy


def _fix_first_rows(nc, dstv, srcv):
    """First-derivative one-sided ends on rows 0 and 63 ([p, 64, 64] views).
    First-end coefficients mirrored at the far end (sign flip there cancels
    in products / makes the BC scale uniform)."""
    for (r0, r1, r2) in ((0, 1, 2), (N - 1, N - 2, N - 3)):
        d = dstv[:, r0:r0 + 1, :]
        f0 = srcv[:, r0:r0 + 1, :]
        f1 = srcv[:, r1:r1 + 1, :]
        f2 = srcv[:, r2:r2 + 1, :]
        nc.vector.scalar_tensor_tensor(d, f0, -3.0, f2, ALU.mult, ALU.subtract)
        nc.vector.scalar_tensor_tensor(d, f1, 4.0, d, ALU.mult, ALU.add)


def _fix_second_rows(nc, dstv, srcv):
    for (r0, r1, r2, r3) in ((0, 1, 2, 3), (N - 1, N - 2, N - 3, N - 4)):
        d = dstv[:, r0:r0 + 1, :]
        f0 = srcv[:, r0:r0 + 1, :]
        f1 = srcv[:, r1:r1 + 1, :]
        f2 = srcv[:, r2:r2 + 1, :]
        f3 = srcv[:, r3:r3 + 1, :]
        nc.vector.scalar_tensor_tensor(d, f0, 2.0, f3, ALU.mult, ALU.subtract)
        nc.vector.scalar_tensor_tensor(d, f1, -5.0, d, ALU.mult, ALU.add)
        nc.vector.scalar_tensor_tensor(d, f2, 4.0, d, ALU.mult, ALU.add)


def _fix_first_cols(nc, dstv, srcv):
    for (c0, c1, c2) in ((0, 1, 2), (N - 1, N - 2, N - 3)):
        d = dstv[:, :, c0:c0 + 1]
        f0 = srcv[:, :, c0:c0 + 1]
        f1 = srcv[:, :, c1:c1 + 1]
        f2 = srcv[:, :, c2:c2 + 1]
        nc.vector.scalar_tensor_tensor(d, f0, -3.0, f2, ALU.mult, ALU.subtract)
        nc.vector.scalar_tensor_tensor(d, f1, 4.0, d, ALU.mult, ALU.add)


def _fix_second_cols(nc, dstv, srcv):
    for (c0, c1, c2, c3) in ((0, 1, 2, 3), (N - 1, N - 2, N - 3, N - 4)):
        d = dstv[:, :, c0:c0 + 1]
        f0 = srcv[:, :, c0:c0 + 1]
        f1 = srcv[:, :, c1:c1 + 1]
        f2 = srcv[:, :, c2:c2 + 1]
        f3 = srcv[:, :, c3:c3 + 1]
        nc.vector.scalar_tensor_tensor(d, f0, 2.0, f3, ALU.mult, ALU.subtract)
        nc.vector.scalar_tensor_tensor(d, f1, -5.0, d, ALU.mult, ALU.add)
        nc.vector.scalar_tensor_tensor(d, f2, 4.0, d, ALU.mult, ALU.add)


def _emit_tile(tc, out_ap, s0, bufs, shared, last_tile):
    """Emit one 128-sample tile starting at sample s0 (within this core).
    x1s holds -C4*x1 (prescaled by ACT, in place over the loaded x1)."""
    nc = tc.nc
    x0b, x1s, a0c, p0c, q0, pj0, pj1, q1, v, bc1, bc2c = bufs
    x1d, w4, tjq, resf, x1f, x_ap = shared

    x0v = x0b[:].rearrange("p (h w) -> p h w", h=N)

    # ---- x1 load (HWDGE, fp32 -> shared staging; SWDGE descriptor-ring
    # traffic interferes with DVE streaming, HWDGE does not) ---------------
    nc.sync.dma_start(
        out=x1f[:],
        in_=x_ap[s0:s0 + P, 1].rearrange("s h w -> s (h w)"))

    # ---- ACT prescales (cast fp32->bf16 fused into the scale) -----------
    # x1' = -C4 * x1; x1d = 2*x1'; w4 = 4*(x0+1)
    nc.scalar.activation(x1s[:], x1f[:], COPY, bias=0.0, scale=-C4)
    x1v = x1s[:].rearrange("p (h w) -> p h w", h=N)
    nc.scalar.activation(x1d[:], x1s[:], COPY, bias=0.0, scale=2.0)
    nc.scalar.activation(w4[:], x0b[:], COPY, bias=4.0, scale=4.0)

    # ---- j-direction stencils into +1-shifted buffers (APs all even) ----
    # tjq[k] = x1'[k] + x1'[k-2]  (t_j shifted +1); pj1[k] = P1'[k-1]
    nc.vector.tensor_add(tjq[:, 2:G], x1s[:, 2:G], x1s[:, 0:G - 2])
    nc.vector.tensor_sub(pj1[:, 2:G], x1s[:, 2:G], x1s[:, 0:G - 2])
    pj1n = pj1[:, 1:1 + G].rearrange("p (h w) -> p h w", h=N)
    _fix_first_cols(nc, pj1n, x1v)
    # q1 = t_j - 2*x1' at natural positions (parity crossing: odd APs, 1x).
    # GpSimd would run this in parallel, but a streaming GpSimd tensor op
    # slows concurrent DVE ops ~4x (SBUF interference), so DVE-1x is faster.
    nc.vector.tensor_sub(q1[:, 1:G - 1], tjq[:, 2:G], x1d[:, 1:G - 1])

    nc.vector.tensor_sub(pj0[:, 2:G], x0b[:, 2:G], x0b[:, 0:G - 2])
    pj0n = pj0[:, 1:1 + G].rearrange("p (h w) -> p h w", h=N)
    _fix_first_cols(nc, pj0n, x0v)
    # V = A1 * P1' on the +1-shifted grid (all-even APs keep 2x mode)
    nc.vector.tensor_mul(v[:], pj0[:], pj1[:])

    # ---- i-direction stencils ------------------------------------------
    nc.vector.tensor_sub(p0c[:, N:G - N], x1s[:, 2 * N:G], x1s[:, 0:G - 2 * N])
    p0v = p0c[:].rearrange("p (h w) -> p h w", h=N)
    _fix_first_rows(nc, p0v, x1v)
    nc.vector.tensor_sub(a0c[:, N:G - N], x0b[:, 2 * N:G], x0b[:, 0:G - 2 * N])
    a0v = a0c[:].rearrange("p (h w) -> p h w", h=N)
    _fix_first_rows(nc, a0v, x0v)

    # BC ch1: rows 0,63 of P0' -> fp32 bc1, stored dense (2 rows/sample).
    nc.scalar.activation(bc1[:], p0v[:, 0:N:N - 1, :], COPY,
                         bias=0.0, scale=BC1_SCALE)
    nc.sync.dma_start(out=out_ap[s0:s0 + P, 1, 0:N:N - 1, :], in_=bc1[:])
    # BC ch2: cols 0,63 of P1' compacted to fp32 [P,64,2]; only those two
    # columns are stored (the rest of the plane relies on the runtime's
    # pre-zeroed output buffer, same as ch1's interior rows).
    nc.scalar.activation(bc2c[:], pj1n[:, :, 0:N:N - 1], COPY,
                         bias=0.0, scale=BC2_SCALE)
    for c in (0, 1):
        nc.sync.dma_start(out=out_ap[s0:s0 + P, 2, :, c * (N - 1)],
                          in_=bc2c[:, :, c])

    # U = A0 * P0' (in place over a0c)
    nc.vector.tensor_mul(a0c[:], a0c[:], p0c[:])

    # t_i and q0 (x1' channel)
    nc.vector.tensor_add(q0[:, N:G - N], x1s[:, 2 * N:G], x1s[:, 0:G - 2 * N])
    nc.vector.tensor_sub(q0[:, N:G - N], q0[:, N:G - N], x1d[:, N:G - N])
    q0v = q0[:].rearrange("p (h w) -> p h w", h=N)
    _fix_second_rows(nc, q0v, x1v)
    q1v = q1[:].rearrange("p (h w) -> p h w", h=N)
    _fix_second_cols(nc, q1v, x1v)

    # ---- combine (in-place over q0) ------------------------------------
    nc.vector.tensor_add(q0[:], q0[:], q1[:])   # S2'
    nc.vector.tensor_mul(q0[:], w4[:], q0[:])   # 4(x0+1)*S2'
    nc.vector.tensor_add(q0[:], q0[:], a0c[:])  # + A0*P0'
    # + A1*P1' back on the natural grid (parity crossing: odd src, 1x)
    nc.vector.tensor_add(q0[:], q0[:], v[:, 1:1 + G])

    # source-term corners: res[0:8,0:8] -= 10 ; res[56:64,56:64] += 10
    nc.scalar.activation(q0v[:, 0:8, 0:8], q0v[:, 0:8, 0:8], COPY,
                         bias=-10.0, scale=1.0)
    nc.scalar.activation(q0v[:, N - 8:N, N - 8:N], q0v[:, N - 8:N, N - 8:N],
                         COPY, bias=10.0, scale=1.0)
    # cast bf16 -> fp32 on ACT, store via HWDGE (keeps SWDGE descriptor
    # generation off the Q7 cores that run the GpSimd tensor ops)
    nc.scalar.activation(resf[:], q0[:], COPY, bias=0.0, scale=1.0)
    nc.scalar.dma_start(
        out=out_ap[s0:s0 + P, 0].rearrange("s h w -> s (h w)"), in_=resf[:])


_WAITSPLIT_N = [0]


def _split_excess_waits(nc, max_waits=1):
    """Engine compute-instruction ISA structs hold only one sync-wait slot;
    Tile can assign several at cross-engine join points ("Too many sync wait
    commands" at codegen).  Move all but one wait onto InstNoOp carriers
    inserted just before, on the same engine."""
    keep = (mybir.InstEventSemaphore,
            mybir.InstCall, mybir.InstUnconditionalBranch, mybir.InstNoOp,
            mybir.InstRegisterMove, mybir.InstISA)
    for f in nc.m.functions:
        for b in f.blocks:
            new_insts = []
            for inst in b.instructions:
                si = inst.sync_info
                if (si is not None and si.on_wait and len(si.on_wait) > max_waits
                        and not isinstance(inst, keep)
                        and getattr(inst, "engine", None) is not None):
                    waits = list(si.on_wait)
                    excess, rest = waits[:-max_waits], waits[-max_waits:]
                    for w in excess:
                        _WAITSPLIT_N[0] += 1
                        nop = mybir.InstNoOp(
                            name=f"waitsplit_{_WAITSPLIT_N[0]}",
                            engine=inst.engine,
                            sync_info=mybir.SyncInfo(on_wait=[w], on_update=[]),
                            bass_nofuse=True,
                        )
                        new_insts.append(nop)
                    inst.sync_info = mybir.SyncInfo(on_wait=rest,
                                                    on_update=list(si.on_update))
                new_insts.append(inst)
            b.instructions = new_insts


def build_bass(split_waits=True):
    nc = bass.Bass()
    x = nc.declare_dram_parameter("x", [S_PER_CORE, 2, N, N], F32,
                                  isOutput=False)
    out = nc.declare_dram_parameter("out", [S_PER_CORE, 3, N, N], F32,
                                    isOutput=True)
    with tile.TileContext(nc) as tc:
        with ExitStack() as ctx:
            pool = ctx.enter_context(tc.tile_pool(name="scratch", bufs=1))
            n_tiles = S_PER_CORE // P

            # shared across tiles (short lifetimes / serialization is cheap)
            x1d = pool.tile([P, G], BF16, tag="x1d", name="x1d")
            w4 = pool.tile([P, G], BF16, tag="w4", name="w4")
            tjq = pool.tile([P, G + 2], BF16, tag="tjq", name="tjq")
            resf = pool.tile([P, G], F32, tag="resf", name="resf")
            x1f = pool.tile([P, G], F32, tag="x1f", name="x1f")
            shared = (x1d, w4, tjq, resf, x1f, x[:])

            sets = []
            for t in range(2):
                x0b = pool.tile([P, G], BF16, tag=f"x0b{t}", name=f"x0b{t}")
                x1s = pool.tile([P, G], BF16, tag=f"x1s{t}", name=f"x1s{t}")
                a0c = pool.tile([P, G], BF16, tag=f"a0c{t}", name=f"a0c{t}")
                p0c = pool.tile([P, G], BF16, tag=f"p0c{t}", name=f"p0c{t}")
                q0 = pool.tile([P, G], BF16, tag=f"q0_{t}", name=f"q0_{t}")
                pj0 = pool.tile([P, G + 2], BF16, tag=f"pj0_{t}",
                                name=f"pj0_{t}")
                pj1 = pool.tile([P, G + 2], BF16, tag=f"pj1_{t}",
                                name=f"pj1_{t}")
                q1 = pool.tile([P, G], BF16, tag=f"q1_{t}", name=f"q1_{t}")
                v = pool.tile([P, G + 2], BF16, tag=f"v{t}", name=f"v{t}")
                bc1 = pool.tile([P, 2, N], F32, tag=f"bc1_{t}", name=f"bc1_{t}")
                bc2c = pool.tile([P, N, 2], F32, tag=f"bc2c{t}",
                                 name=f"bc2c{t}")
                sets.append((x0b, x1s, a0c, p0c, q0, pj0, pj1, q1, v, bc1,
                             bc2c))

            # prefetch the x0 channel up front (fp32 -> bf16 SWDGE cast,
            # finishes before the DVE chain is in full swing); x1 is loaded
            # per tile via HWDGE into the fp32 staging buffer
            for it in range(n_tiles):
                nc.gpsimd.dma_start(
                    out=sets[it % 2][0][:],
                    in_=x[:][it * P:(it + 1) * P, 0].rearrange(
                        "s h w -> s (h w)"))

            for it in range(n_tiles):
                _emit_tile(tc, out[:], it * P, sets[it % 2], shared,
                           last_tile=(it == n_tiles - 1))
    if split_waits:
        _split_excess_waits(nc)
    return nc


_NC = None


def _get_nc():
    global _NC
    if _NC is None:
        _NC = build_bass()
    return _NC


def _axon_device_reset():
    """Recover a wedged NeuronCore (NRT_EXEC_UNIT_UNRECOVERABLE) via the
    axon client's reset entry point."""
    try:
        import ctypes

        import jax

        jax.devices()
        lib = ctypes.CDLL("/opt/axon/libaxon_pjrt.so")
        lib.axon_reset.restype = ctypes.c_int64
        return int(lib.axon_reset()) == 0
    except Exception:
        return False


def kernel(x0_pred, compute_bc=1, **_):
    from concourse.bass_utils import run_bass_kernel_spmd

    x = np.ascontiguousarray(np.asarray(x0_pred), dtype=np.float32)
    assert x.shape == (B, 2, N, N), x.shape
    nc = _get_nc()
    shards = x.reshape(N_CORES, S_PER_CORE, 2, N, N)
    in_maps = [{"x": shards[i]} for i in range(N_CORES)]
    try:
        res = run_bass_kernel_spmd(nc, in_maps, list(range(N_CORES)))
    except Exception:
        if not _axon_device_reset():
            raise
        res = run_bass_kernel_spmd(nc, in_maps, list(range(N_CORES)))
    full = np.concatenate([res.results[i]["out"] for i in range(N_CORES)],
                          axis=0)
    if not int(np.asarray(compute_bc)):
        return full[:, :1]
    return full
